# revision 74
# baseline (speedup 1.0000x reference)
"""AdaptiveBlockSelfAttention Trainium2 kernel (8 NeuronCores), v2.

Math (per batch b, channel c, in blocked layout):
  X_c = x[b,c] unfolded to a 256x256 matrix [n, p] (n = 16x16 block index,
        p = 16x16 pixel-in-block index).
  Q/K/V = per-pixel channel mixing (1x1 conv) of X across c.
  T = K^T Q  (contract n)            -> [q, p]   (= S^T of the reference)
  E = exp(T / sqrt(C))               (no max-subtraction; logits are small)
  U' = E^T @ [V | 1]                 -> [p, 0:256]=numerator, [p,256]=denom
  O = U'[:, :256] / U'[:, 256:]      rows of O are output blocks n'=p
  x1 = X + O ; out = x1 + FFN(x1)    FFN mixes channels per pixel.

Sharding: core k = (b = k//2, h = k%2); attention over 96 channels x full
image, FFN over the core's token half x all 192 channels, with chunked
2-core AllGathers of x1 overlapping the attention phase.

v2 changes vs v1 (cost-model estimate 619us -> 481us, rel err 1.18e-2):
  - Phase-1 PSUM evacuation runs as [96, 1024] copies from 2-bank PSUM
    tiles (half the per-copy fixed overhead on DVE/ACT, which pace the
    phase). This same change LOST 6us in an earlier configuration --
    the trade flipped once the spill WAW chain and the bf16 load flood
    were removed and evac became the sole pacer.
  - V tile is ungapped (i, ch, p) so V spill reads keep the source's
    1KB contiguous runs (1x DMA descriptor rate instead of the 2x
    sub-512B penalty); the softmax denominator is accumulated by two
    N=1 matmuls against a ones column (PE has slack in phase 2, DMA
    does not) -- numerically identical.
  - The FFN residual x is loaded just-in-time inside the phase-3
    software pipeline (phase-3 DMA runs at ~30%), not pre-cached in
    phase 1: phase 1 sheds 12.6MB of bf16 loads and the early DMA
    flood, leaving it paced by the PSUM evacuation alone.
  - Spill tensors laid out (n2, j, c, ip) / (in1, c, p): each spill
    write's strided footprint is compact and disjoint across token
    groups, so the dependency tracker no longer fabricates a WAW chain
    between spill DMAs (that chain had paced all of phase 1 at ~2us
    per spill); qk spill reads also gain 2KB descriptor runs.
  - QKV projections run as DoubleRow fp8 matmuls: x is cast-loaded
    (SWDGE) into channel-pair e4m3 tiles [97, 2, t], weights host-side
    pair-interleaved e4m3 [97, 2, 96] (+zero pad row 193); one DR
    matmul replaces two bf16 accumulation passes per (tile, proj).
  - Spill DMAs issue on sync/scalar HWDGE queues, NOT gpsimd: their
    embedded evac-waits were head-of-line blocking the next groups'
    cast-loads on the Pool queue (-46us).
  - The cross-core exchange carries O (attention output, |O|~0.3), not
    x1: the residual X is added from SBUF in phase 3. O is quantized to
    fp8 e3m4 (x1s/x1gp/to0/to1), halving exchange DMA + AllGather wire
    bytes for ~0.4% extra error.
  - Q,K spilled to DRAM in fp8 e4m3, layout (c, n2, j, ip): reads land
    [block-pair partitions x 1KB runs], writes keep 512B runs; the
    scores matmul runs in DoubleRow fp8 (contraction 256 in one pass,
    half the bf16 column-streams). V spilled in fp8 e3m4 (c, t).
  - Phase 3 is software-pipelined (loads+adds 2 groups ahead, h-matmuls
    1 ahead) so the adds(DVE)->h(PE)->gelu(ACT)->y(PE)->stt(DVE) chain
    never stalls an engine: PE runs at ~100% through the FFN.
  - Fewer/larger DMAs (2-t2 batches, merged u/m x1s writes, merged out
    writes); one batched exp per channel; FFN tail fused into one
    scalar_tensor_tensor (y + bias + residual).
  - strict_bb_all_engine_barrier() between the phases: the spill/x1s
    writes use runtime DMA offsets that the Tile dependency tracker
    cannot pair with the next phase's static reads (verified races
    without it -- dep_tracking_offset aliases are NOT honored for
    these DMA writes).
"""
import os
os.environ.setdefault("MYCRO_LOCAL_CACHE", "1")
import numpy as np
import ml_dtypes
import concourse.bass as bass
import concourse.bacc as bacc
import concourse.tile as tile
import concourse.mybir as mybir
from concourse.bass_utils import run_bass_kernel_spmd

F32 = mybir.dt.float32
BF16 = mybir.dt.bfloat16
E4 = mybir.dt.float8e4
E3 = mybir.dt.float8e3
AF = mybir.ActivationFunctionType
DR = mybir.MatmulPerfMode.DoubleRow

B, C, H, W = 4, 192, 256, 256
NPIX = H * W            # 65536 tokens per batch
HNP = NPIX // 2
CH = C // 2             # 96 channels per core
HID = 384
TT = 512                # FFN token tile
T2 = 1024               # phase-1 token tile
SCALE = 1.0 / float(np.sqrt(C))
NCHUNK = 2              # AllGather chunks over channels
CCH = CH // NCHUNK

_NC_CACHE = {}


def build_nc(sim=False, dbg=False):
    nc = bacc.Bacc("TRN2", target_bir_lowering=False, debug=False,
                   num_devices=1 if sim else 8)

    x = nc.dram_tensor("x", [C + 2, NPIX], BF16, kind="ExternalInput")
    wq = nc.dram_tensor("wq", [97, 2 * CH], E4, kind="ExternalInput")
    wk = nc.dram_tensor("wk", [97, 2 * CH], E4, kind="ExternalInput")
    wv = nc.dram_tensor("wv", [97, 2 * CH], E4, kind="ExternalInput")
    wf1 = nc.dram_tensor("wf1", [C, HID], BF16, kind="ExternalInput")
    bf1c = nc.dram_tensor("bf1c", [HID, 1], F32, kind="ExternalInput")
    wf2 = nc.dram_tensor("wf2", [HID, C], BF16, kind="ExternalInput")
    bf2c = nc.dram_tensor("bf2c", [C, 1], F32, kind="ExternalInput")
    dyn = nc.dram_tensor("dyn", [1, 9], mybir.dt.uint32, kind="ExternalInput")
    out = nc.dram_tensor("out", [C, HNP], BF16, kind="ExternalOutput")

    # fp8 spills, natural local-channel rows.
    # qks layout per channel: (n2 128 block-pairs, j {q,k}, ip 512 tokens)
    qks = nc.dram_tensor("qks", [1, CH * 2 * NPIX], E4)
    vs = nc.dram_tensor("vs", [1, CH * NPIX], E3)  # plain (c,t)
    # x1s flat: block0 = peer-destined halves, block1 = own halves
    x1s = nc.dram_tensor("x1s", [1, 2 * CH * HNP], E3)
    x1gp = nc.dram_tensor("x1gp", [NCHUNK * 2 * CCH, HNP], E3)
    if dbg:
        qksb = nc.dram_tensor("qksb", [1, CH * 2 * NPIX], BF16,
                              kind="ExternalOutput")
        vsb = nc.dram_tensor("vsb", [1, CH * NPIX], BF16,
                             kind="ExternalOutput")
        x1sb = nc.dram_tensor("x1sb", [1, 2 * CH * HNP], BF16,
                              kind="ExternalOutput")
        x1gpb = nc.dram_tensor("x1gpb", [NCHUNK * 2 * CCH, HNP], BF16,
                               kind="ExternalOutput")
        xfb = nc.dram_tensor("xfb", [2 * C, HNP], BF16,
                             kind="ExternalOutput")

    xa_v = x.ap()
    xa_p = x.ap().rearrange("(r k) t -> r k t", k=2)   # 97 pair-rows
    # layout (n2, j, c, ip): per-write footprints are compact and
    # disjoint across token groups, so the dep tracker creates no false
    # WAW chain between spill DMAs (that chain paced all of phase 1).
    qks_wj = [qks.ap().rearrange("o (n2 j c ip) -> (o j) c n2 ip",
                                 n2=128, j=2, ip=512)[j] for j in range(2)]
    # read view: [cgroup, n2(part), j, (c ip)]
    qks_r = qks.ap().rearrange("o (n2 j cg cip) -> cg (o n2) j cip",
                               n2=128, j=2, cip=4 * 512)
    # vs layout (in1, c, p), same reasoning
    vs_w = vs.ap().rearrange("o (q c p) -> (o c) q p", c=CH, p=256)
    # read view: [cgroup, i, n1(part), (c p)]
    vs_r = vs.ap().rearrange("o (i n1 cg cip) -> cg i (o n1) cip",
                             i=2, n1=128, cip=4 * 256)
    x1s_r = x1s.ap().rearrange("o (r t) -> (o r) t", t=HNP)
    # per-channel-row view with the output-block partition dim leading:
    # row = block*CH + channel; [n 128, row, p 256]
    x1s_n = x1s.ap().rearrange("o (r n p) -> n (o r) p", n=128, p=256)
    x1gp_g = x1gp.ap().rearrange("(g rc) t -> g rc t", g=NCHUNK)
    x1gp_f = x1gp.ap()
    out_v = out.ap().rearrange("(g c) (w t) -> c g w t", g=2, t=TT)

    def with_track(a, off):
        return bass.AP(tensor=a.tensor, offset=a.offset, ap=a.ap,
                       const_val=a.const_val,
                       runtime_checks=a.runtime_checks,
                       dep_tracking_offset=off)

    with tile.TileContext(nc) as tc:
        # ---- persistent weights + own-half x cache ----
        with tc.tile_pool(name="wpool", bufs=1) as wp:
            ones_t = wp.tile([128, 1], E3, name="ones", tag="ones")
            nc.gpsimd.memset(ones_t[:], 1.0)
            w_t = {}
            for nm, wt in (("q", wq), ("k", wk), ("v", wv)):
                w_t[nm] = wp.tile([97, 2 * CH], E4, name=f"wt{nm}",
                                  tag=f"wt{nm}")
                nc.sync.dma_start(w_t[nm][:], wt.ap()[:, :])
            wf1_a = wp.tile([CH, HID], BF16, name="wf1a", tag="wf1a")
            wf1_b = wp.tile([CH, HID], BF16, name="wf1b", tag="wf1b")
            wf2_h = [wp.tile([128, C], BF16, name=f"wf2{hc}", tag=f"wf2{hc}")
                     for hc in range(3)]
            bf1_t = [wp.tile([128, 1], F32, name=f"bf1{hc}", tag=f"bf1{hc}")
                     for hc in range(3)]
            bf2_t = [wp.tile([CH, 1], F32, name=f"bf2{cc}", tag=f"bf2{cc}")
                     for cc in range(2)]

            def load_ffn_weights():
                nc.sync.dma_start(wf1_a[:], wf1.ap()[0:CH, :])
                nc.sync.dma_start(wf1_b[:], wf1.ap()[CH:C, :])
                for hc in range(3):
                    nc.sync.dma_start(wf2_h[hc][:],
                                      wf2.ap()[hc * 128:(hc + 1) * 128, :])
                    nc.sync.dma_start(bf1_t[hc][:],
                                      bf1c.ap()[hc * 128:(hc + 1) * 128, :])
                for cc in range(2):
                    nc.sync.dma_start(bf2_t[cc][:],
                                      bf2c.ap()[cc * CH:(cc + 1) * CH, :])
            dyn_sb = wp.tile([1, 9], mybir.dt.uint32, name="dyn", tag="dyn")
            nc.sync.dma_start(dyn_sb[:], dyn.ap()[:, :])
            # o_m[m]: x1s row base (block*CH) for image-half m, in row units
            o_m = [nc.values_load(dyn_sb[0:1, i:i + 1], min_val=0,
                                  max_val=CH,
                                  skip_runtime_bounds_check=True)
                   for i in range(2)]
            pb0 = nc.values_load(dyn_sb[0:1, 2:3], min_val=0, max_val=CCH,
                                 skip_runtime_bounds_check=True)
            tv = nc.values_load(dyn_sb[0:1, 3:4], min_val=0, max_val=HNP,
                                skip_runtime_bounds_check=True)
            tv2 = nc.values_load(dyn_sb[0:1, 4:5], min_val=0, max_val=HNP,
                                 skip_runtime_bounds_check=True)
            # same offsets in block-pair (512-token) units for the qk spill
            tvn = nc.values_load(dyn_sb[0:1, 5:6], min_val=0, max_val=64,
                                 skip_runtime_bounds_check=True)
            tv2n = nc.values_load(dyn_sb[0:1, 6:7], min_val=0, max_val=64,
                                  skip_runtime_bounds_check=True)
            # offsets in 256-token units for the vs spill
            tvp = nc.values_load(dyn_sb[0:1, 7:8], min_val=0, max_val=128,
                                 skip_runtime_bounds_check=True)
            tv2p = nc.values_load(dyn_sb[0:1, 8:9], min_val=0, max_val=128,
                                  skip_runtime_bounds_check=True)


            # ---- phase 1: QKV projections ----
            # token order: own half (t2 0..31) then peer half; 2-t2 batches
            with tc.tile_pool(name="px", bufs=8) as px, \
                 tc.tile_pool(name="pev", bufs=6) as pev, \
                 tc.tile_pool(name="psP", bufs=4, space="PSUM") as psP:
                for g2 in range(32):
                    own = g2 < 16
                    base = tv if own else tv2
                    basen = tvn if own else tv2n
                    basep = tvp if own else tv2p
                    goff = (g2 % 16) * (2 * T2)
                    o = base + goff
                    on = basen + (g2 % 16) * 4
                    op = basep + (g2 % 16) * 8
                    xe = px.tile([97, 2 * 2 * T2], E4, name="xe", tag="xe")
                    xev = xe[:].rearrange("r (k t) -> r k t", k=2)
                    srce = with_track(xa_p[:, :, bass.ds(o, 2 * T2)], goff)
                    nc.gpsimd.dma_start(xev, srce)
                    cqk = pev.tile([CH, 2 * 2 * T2], E4, name="cqk",
                                   tag="cqk")
                    cqkv = cqk[:].rearrange("c (j n2 ip) -> c j n2 ip",
                                            j=2, ip=512)
                    cv = pev.tile([CH, 2 * T2], E3, name="cv", tag="cv")
                    for hh in range(2):
                        hhs = slice(hh * T2, (hh + 1) * T2)
                        for j, nm in enumerate(("q", "k", "v")):
                            ps = psP.tile([CH, T2], F32, name="pp", tag="pp")
                            for q in range(2):
                                hs = slice(hh * T2 + q * TT,
                                           hh * T2 + (q + 1) * TT)
                                nc.tensor.matmul(
                                    ps[:, q * TT:(q + 1) * TT],
                                    w_t[nm][:].rearrange(
                                        "r (k m) -> r k m", k=2),
                                    xev[:, :, hs], start=True, stop=True,
                                    perf_mode=DR)
                            if j == 2:
                                dst_ = cv[:, hhs]
                            elif j == 1:
                                dst_ = cqk[:, 2048 + hh * T2:
                                           2048 + (hh + 1) * T2]
                            else:
                                dst_ = cqk[:, hh * T2:(hh + 1) * T2]
                            if (3 * hh + j) % 2 == 0:
                                nc.vector.tensor_copy(dst_, ps[:])
                            else:
                                nc.scalar.copy(dst_, ps[:])
                    # spills on HWDGE queues (sync/scalar): keeping them off
                    # the Pool queue stops their embedded evac-waits from
                    # head-of-line blocking the next groups' cast-loads.
                    for j in range(2):
                        dstq = with_track(
                            qks_wj[j][:, bass.ds(on, 4), :],
                            g2 * 4 * 2 * CH * 512)
                        nc.sync.dma_start(dstq, cqkv[:, j, :, :])
                    dstv = with_track(
                        vs_w[:, bass.ds(op, 8), :], g2 * 8 * CH * 256)
                    nc.scalar.dma_start(
                        dstv, cv[:].rearrange("c (q p) -> c q p", p=256))

            if dbg:
                nc.gpsimd.dma_start(qksb.ap()[0], qks.ap()[0])
                nc.gpsimd.dma_start(vsb.ap()[0], vs.ap()[0])


            # the spill writes use dynamic offsets the dependency tracker
            # cannot pair with phase 2's static reads -- hard barrier.
            tc.strict_bb_all_engine_barrier()

            # ---- phase 2: per-channel attention (+ overlapped AllGather) ---
            load_ffn_weights()
            with tc.tile_pool(name="aq", bufs=6) as aq, \
                 tc.tile_pool(name="ao", bufs=6) as ao, \
                 tc.tile_pool(name="ar", bufs=8) as ar, \
                 tc.tile_pool(name="psT", bufs=4, space="PSUM") as psT, \
                 tc.tile_pool(name="psU", bufs=4, space="PSUM") as psU:
                for cp2 in range(CH // 4):
                    # 4 consecutive channels per load
                    csl = slice(4 * cp2, 4 * cp2 + 4)
                    qkt = aq.tile([128, 4096], E4, name="qkt", tag="qkt")
                    qktv = qkt[:].rearrange("n (j ch w p) -> n j ch w p",
                                            ch=4, j=2, p=256)
                    nc.sync.dma_start(
                        qkt[:].rearrange("n (j cip) -> n j cip", j=2),
                        qks_r[cp2])
                    # ungapped V tile (i, ch, p): loads keep the source's
                    # 1KB contiguous runs (1x DMA rate); the softmax
                    # denominator comes from two N=1 matmuls vs a ones
                    # column instead of a gap-embedded ones column.
                    vv = aq.tile([128, 2048], E3, name="vv", tag="vv")
                    vvv = vv[:].rearrange("n (i ch p) -> n i ch p",
                                          ch=4, p=256)
                    for i in range(2):
                        nc.sync.dma_start(
                            vv[:, i * 1024:(i + 1) * 1024], vs_r[cp2, i])
                    for pr in range(2):
                        c0 = 4 * cp2 + 2 * pr  # first of 2 adjacent channels
                        ob4 = ao.tile([128, 1024], E3, name="ob4",
                                      tag="ob4")
                        ob4v = ob4[:].rearrange("n (u m p) -> n u m p",
                                                u=2, p=256)
                        for uu in range(2):
                            ch = 2 * pr + uu
                            tps = psT.tile([128, 512], F32, name="t",
                                           tag="t")
                            for j in range(2):
                                nc.tensor.matmul(
                                    tps[:, j * 256:(j + 1) * 256],
                                    qktv[:, 1, ch, :,
                                         j * 128:(j + 1) * 128],
                                    qktv[:, 0, ch, :, :],
                                    start=True, stop=True, perf_mode=DR)
                            te = ar.tile([128, 512], BF16, name="te",
                                         tag="te")
                            nc.scalar.activation(te[:], tps[:], AF.Exp,
                                                 scale=SCALE)
                            for m in range(2):
                                ups = psU.tile([128, 272], F32, name="u",
                                               tag="u")
                                for j in range(2):
                                    nc.tensor.matmul(
                                        ups[:, 0:256],
                                        te[:, j * 256 + m * 128:
                                           j * 256 + (m + 1) * 128],
                                        vvv[:, j, ch, :],
                                        start=(j == 0), stop=False,
                                        skip_group_check=True)
                                for j in range(2):
                                    nc.tensor.matmul(
                                        ups[:, 256:257],
                                        te[:, j * 256 + m * 128:
                                           j * 256 + (m + 1) * 128],
                                        ones_t[:],
                                        start=False, stop=(j == 1),
                                        skip_group_check=True)
                                rc = ar.tile([128, 1], F32, name="rc",
                                             tag="rc")
                                nc.vector.reciprocal(rc[:], ups[:, 256:257])
                                nc.vector.tensor_scalar_mul(
                                    ob4v[:, uu, m, :],
                                    ups[:, 0:256], rc[:])
                        # one DMA per image-half: 2 channels x [128, 256]
                        for m in range(2):
                            dst = x1s_n[:, bass.ds(o_m[m] + c0, 2), :]
                            dst = with_track(dst, c0 * HNP)
                            weng = nc.sync if m == 0 else nc.gpsimd
                            weng.dma_start(dst, ob4v[:, :, m, :])
                    # chunked exchange as soon as a channel group is done
                    cend = 4 * cp2 + 4
                    if cend % CCH == 0:
                        g = cend // CCH - 1
                        gsl = slice(g * CCH, (g + 1) * CCH)
                        src = x1s_r[gsl, :]
                        dst = x1gp_f[g * 2 * CCH:(g + 1) * 2 * CCH, :]
                        if sim:
                            dv = dst.rearrange("(r c) t -> r c t", r=2)
                            nc.sync.dma_start(dv[0], src)
                            nc.sync.dma_start(dv[1], src)
                        else:
                            nc.gpsimd.collective_compute(
                                "AllGather", mybir.AluOpType.bypass,
                                replica_groups=[[0, 1], [2, 3], [4, 5],
                                                [6, 7]],
                                ins=[src], outs=[dst],
                            )

            if dbg:
                nc.gpsimd.dma_start(x1sb.ap()[0], x1s.ap()[0])
                nc.gpsimd.dma_start(x1gpb.ap()[:, :], x1gp.ap()[:, :])

            # x1s writes use dynamic offsets; barrier before the FFN reads
            tc.strict_bb_all_engine_barrier()

            # ---- phase 3: FFN on my token half (2-TT groups) ----
            # software-pipelined: loads+adds 2 groups ahead, h-matmuls 1
            # group ahead, so no engine stalls on the cross-engine chain
            # adds(DVE/Pool) -> h(PE) -> gelu(ACT) -> y(PE) -> stt(DVE).
            NTG = NPIX // 2 // T2  # 32 groups of 1024 tokens
            with tc.tile_pool(name="fx", bufs=4) as fx, \
                 tc.tile_pool(name="fh", bufs=6) as fh, \
                 tc.tile_pool(name="fo", bufs=4) as fo, \
                 tc.tile_pool(name="psH", bufs=6, space="PSUM") as psH, \
                 tc.tile_pool(name="psY", bufs=2, space="PSUM") as psY:

                def p3_loads(tg):
                    toff = tg * T2
                    to0 = fx.tile([CH, T2], E3, name="to0", tag="to0")
                    src0 = with_track(
                        x1s_r[CH:2 * CH, toff:toff + T2], toff)
                    nc.sync.dma_start(to0[:], src0)
                    to1 = fx.tile([CH, T2], E3, name="to1", tag="to1")
                    for g in range(NCHUNK):
                        eng = nc.sync if g % 2 == 0 else nc.gpsimd
                        eng.dma_start(
                            to1[g * CCH:(g + 1) * CCH, :],
                            x1gp_g[g, bass.ds(pb0, CCH), toff:toff + T2])
                    xr0 = fx.tile([CH, T2], BF16, name="xr0", tag="xr0")
                    nc.sync.dma_start(
                        xr0[:], xa_v[0:CH, bass.ds(tv + toff, T2)])
                    xr1 = fx.tile([CH, T2], BF16, name="xr1", tag="xr1")
                    nc.scalar.dma_start(
                        xr1[:], xa_v[CH:C, bass.ds(tv + toff, T2)])
                    xf0 = fx.tile([CH, T2], BF16, name="xf0", tag="xf0")
                    nc.vector.tensor_add(xf0[:], to0[:], xr0[:])
                    xf1 = fx.tile([CH, T2], BF16, name="xf1", tag="xf1")
                    nc.gpsimd.tensor_add(xf1[:], to1[:], xr1[:])
                    if dbg:
                        for ti, t_ in ((0, to0), (1, to1), (2, xf0), (3, xf1)):
                            nc.gpsimd.dma_start(
                                xfb.ap()[ti * CH:(ti + 1) * CH,
                                         toff:toff + T2], t_[:])
                    return xf0, xf1

                def p3_h(tg, xfs):
                    xf0, xf1 = xfs
                    hsb = []
                    for s in range(2):
                        ssl = slice(s * TT, (s + 1) * TT)
                        for hc in range(3):
                            hcs = slice(hc * 128, (hc + 1) * 128)
                            hps = psH.tile([128, TT], F32, name="h", tag="h")
                            nc.tensor.matmul(hps[:], wf1_a[:, hcs],
                                             xf0[:, ssl],
                                             start=True, stop=False)
                            nc.tensor.matmul(hps[:], wf1_b[:, hcs],
                                             xf1[:, ssl],
                                             start=False, stop=True)
                            th = fh.tile([128, TT], BF16, name=f"h{hc}",
                                         tag=f"h{hc}")
                            nc.scalar.activation(th[:], hps[:], AF.Gelu,
                                                 bias=bf1_t[hc][:])
                            hsb.append(th)
                    return hsb

                def p3_tail(tg, xfs, hsb):
                    xf0, xf1 = xfs
                    oo = fo.tile([CH, 2 * 2 * TT], BF16, name="oo", tag="oo")
                    oov = oo[:].rearrange("c (g s t) -> c g s t", g=2, s=2)
                    for s in range(2):
                        ssl = slice(s * TT, (s + 1) * TT)
                        for cc in range(2):
                            ccs = slice(cc * CH, (cc + 1) * CH)
                            yps = psY.tile([CH, TT], F32, name="y", tag="y")
                            for hc in range(3):
                                nc.tensor.matmul(yps[:],
                                                 wf2_h[hc][:, ccs],
                                                 hsb[3 * s + hc][:],
                                                 start=(hc == 0),
                                                 stop=(hc == 2))
                            xf = xf0 if cc == 0 else xf1
                            nc.vector.scalar_tensor_tensor(
                                oov[:, cc, s, :], yps[:], bf2_t[cc][:],
                                xf[:, ssl], mybir.AluOpType.add,
                                mybir.AluOpType.add)
                    dsto = with_track(
                        out_v[:, :, 2 * tg:2 * tg + 2, :], tg * T2)
                    nc.sync.dma_start(dsto, oov)

                xq = {0: p3_loads(0), 1: p3_loads(1)}
                hq = {0: p3_h(0, xq[0])}
                for tg in range(NTG):
                    if tg + 2 < NTG:
                        xq[tg + 2] = p3_loads(tg + 2)
                    if tg + 1 < NTG:
                        hq[tg + 1] = p3_h(tg + 1, xq[tg + 1])
                    p3_tail(tg, xq.pop(tg), hq.pop(tg))
    nc.compile()
    return nc


def _get_nc():
    if "nc" not in _NC_CACHE:
        _NC_CACHE["nc"] = build_nc()
    return _NC_CACHE["nc"]


def _block(x):
    """(B,C,256,256) -> (B,C,65536) blocked token order."""
    Bn, Cn = x.shape[0], x.shape[1]
    return (x.reshape(Bn, Cn, 16, 16, 16, 16)
            .transpose(0, 1, 2, 4, 3, 5)
            .reshape(Bn, Cn, NPIX))


def _unblock(y):
    """(B,C,65536) blocked -> (B,C,256,256)."""
    Bn, Cn = y.shape[0], y.shape[1]
    return (y.reshape(Bn, Cn, 16, 16, 16, 16)
            .transpose(0, 1, 2, 4, 3, 5)
            .reshape(Bn, Cn, H, W))


def prepare_in_maps(x, Wq, bq, Wk, bk, Wv, bv, Wf1, bf1, Wf2, bf2):
    xb = _block(np.asarray(x, np.float32))
    xb_bf = xb.astype(ml_dtypes.bfloat16)
    ones = np.ones((1, NPIX), ml_dtypes.bfloat16)
    wf1_f = np.asarray(Wf1, np.float32)
    wf2_f = np.asarray(Wf2, np.float32)
    bf1_in = np.asarray(bf1, np.float32).reshape(HID, 1)
    bf2_f = np.asarray(bf2, np.float32)
    in_maps = []
    for k in range(8):
        b, h = k // 2, k % 2
        own = slice(h * CH, (h + 1) * CH)
        perm = np.r_[np.arange(h * CH, (h + 1) * CH),
                     np.arange((1 - h) * CH, (2 - h) * CH)]
        zrow = np.zeros((1, NPIX), ml_dtypes.bfloat16)
        x_in = np.concatenate([xb_bf[b][perm], ones, zrow], axis=0)
        wf1_in = np.ascontiguousarray(wf1_f[:, perm].T
                                      ).astype(ml_dtypes.bfloat16)
        wf2_in = np.ascontiguousarray(wf2_f[perm].T
                                      ).astype(ml_dtypes.bfloat16)
        bf2_in = bf2_f[perm].reshape(C, 1)
        # o_m in x1s row units: image-half m lands in block o (0=peer-
        # destined, 1=own); row base = o*CH
        tvv, tvv2 = h * HNP, (1 - h) * HNP
        dyn = np.array([[CH if h == 0 else 0, CH if h == 1 else 0,
                         (1 - h) * CCH, tvv, tvv2,
                         tvv // 512, tvv2 // 512,
                         tvv // 256, tvv2 // 256]], np.uint32)
        m = {"x": np.ascontiguousarray(x_in), "dyn": dyn,
             "wf1": wf1_in, "wf2": wf2_in, "bf1c": bf1_in, "bf2c": bf2_in}
        for nm, Wm, bm in (("wq", Wq, bq), ("wk", Wk, bk), ("wv", Wv, bv)):
            Wm = np.asarray(Wm, np.float32)
            wown = Wm[own]
            bown = np.asarray(bm, np.float32)[own]
            wext = np.concatenate([wown[:, perm].T, bown[None, :],
                                   np.zeros((1, CH), np.float32)], axis=0)
            m[nm] = np.ascontiguousarray(wext.reshape(97, 2 * CH)).astype(
                ml_dtypes.float8_e4m3)
        in_maps.append(m)
    return in_maps


def run(in_maps, trace=False, **kw):
    nc = _get_nc()
    return run_bass_kernel_spmd(nc, in_maps, core_ids=list(range(8)),
                                trace=trace, **kw)


def assemble(results):
    yb = np.empty((B, C, NPIX), np.float32)
    for k in range(8):
        b, h = k // 2, k % 2
        perm = np.r_[np.arange(h * CH, (h + 1) * CH),
                     np.arange((1 - h) * CH, (2 - h) * CH)]
        o = results[k]["out"]
        yb[b, perm, h * HNP:(h + 1) * HNP] = o.astype(np.float32)
    return _unblock(yb)


def kernel(**inputs):
    in_maps = prepare_in_maps(**inputs)
    res = run(in_maps)
    return assemble(res.results)


# revision 76
# speedup vs baseline: 1.0004x; 1.0004x over previous
"""AdaptiveBlockSelfAttention Trainium2 kernel (8 NeuronCores), v2.

Math (per batch b, channel c, in blocked layout):
  X_c = x[b,c] unfolded to a 256x256 matrix [n, p] (n = 16x16 block index,
        p = 16x16 pixel-in-block index).
  Q/K/V = per-pixel channel mixing (1x1 conv) of X across c.
  T = K^T Q  (contract n)            -> [q, p]   (= S^T of the reference)
  E = exp(T / sqrt(C))               (no max-subtraction; logits are small)
  U' = E^T @ [V | 1]                 -> [p, 0:256]=numerator, [p,256]=denom
  O = U'[:, :256] / U'[:, 256:]      rows of O are output blocks n'=p
  x1 = X + O ; out = x1 + FFN(x1)    FFN mixes channels per pixel.

Sharding: core k = (b = k//2, h = k%2); attention over 96 channels x full
image, FFN over the core's token half x all 192 channels, with chunked
2-core AllGathers of x1 overlapping the attention phase.

v2 changes vs v1 (cost-model estimate 619us -> 481us, rel err 1.18e-2):
  - Phase-1 PSUM evacuation runs as [96, 1024] copies from 2-bank PSUM
    tiles (half the per-copy fixed overhead on DVE/ACT, which pace the
    phase). This same change LOST 6us in an earlier configuration --
    the trade flipped once the spill WAW chain and the bf16 load flood
    were removed and evac became the sole pacer.
  - V tile is ungapped (i, ch, p) so V spill reads keep the source's
    1KB contiguous runs (1x DMA descriptor rate instead of the 2x
    sub-512B penalty); the softmax denominator is accumulated by two
    N=1 matmuls against a ones column (PE has slack in phase 2, DMA
    does not) -- numerically identical.
  - The FFN residual x is loaded just-in-time inside the phase-3
    software pipeline (phase-3 DMA runs at ~30%), not pre-cached in
    phase 1: phase 1 sheds 12.6MB of bf16 loads and the early DMA
    flood, leaving it paced by the PSUM evacuation alone.
  - Spill tensors laid out (n2, j, c, ip) / (in1, c, p): each spill
    write's strided footprint is compact and disjoint across token
    groups, so the dependency tracker no longer fabricates a WAW chain
    between spill DMAs (that chain had paced all of phase 1 at ~2us
    per spill); qk spill reads also gain 2KB descriptor runs.
  - QKV projections run as DoubleRow fp8 matmuls: x is cast-loaded
    (SWDGE) into channel-pair e4m3 tiles [97, 2, t], weights host-side
    pair-interleaved e4m3 [97, 2, 96] (+zero pad row 193); one DR
    matmul replaces two bf16 accumulation passes per (tile, proj).
  - Spill DMAs issue on sync/scalar HWDGE queues, NOT gpsimd: their
    embedded evac-waits were head-of-line blocking the next groups'
    cast-loads on the Pool queue (-46us).
  - The cross-core exchange carries O (attention output, |O|~0.3), not
    x1: the residual X is added from SBUF in phase 3. O is quantized to
    fp8 e3m4 (x1s/x1gp/to0/to1), halving exchange DMA + AllGather wire
    bytes for ~0.4% extra error.
  - Q,K spilled to DRAM in fp8 e4m3, layout (c, n2, j, ip): reads land
    [block-pair partitions x 1KB runs], writes keep 512B runs; the
    scores matmul runs in DoubleRow fp8 (contraction 256 in one pass,
    half the bf16 column-streams). V spilled in fp8 e3m4 (c, t).
  - Phase 3 is software-pipelined (loads+adds 2 groups ahead, h-matmuls
    1 ahead) so the adds(DVE)->h(PE)->gelu(ACT)->y(PE)->stt(DVE) chain
    never stalls an engine: PE runs at ~100% through the FFN.
  - Fewer/larger DMAs (2-t2 batches, merged u/m x1s writes, merged out
    writes); one batched exp per channel; FFN tail fused into one
    scalar_tensor_tensor (y + bias + residual).
  - strict_bb_all_engine_barrier() between the phases: the spill/x1s
    writes use runtime DMA offsets that the Tile dependency tracker
    cannot pair with the next phase's static reads (verified races
    without it -- dep_tracking_offset aliases are NOT honored for
    these DMA writes).
"""
import os
os.environ.setdefault("MYCRO_LOCAL_CACHE", "1")
import numpy as np
import ml_dtypes
import concourse.bass as bass
import concourse.bacc as bacc
import concourse.tile as tile
import concourse.mybir as mybir
from concourse.bass_utils import run_bass_kernel_spmd

F32 = mybir.dt.float32
BF16 = mybir.dt.bfloat16
E4 = mybir.dt.float8e4
E3 = mybir.dt.float8e3
AF = mybir.ActivationFunctionType
DR = mybir.MatmulPerfMode.DoubleRow

B, C, H, W = 4, 192, 256, 256
NPIX = H * W            # 65536 tokens per batch
HNP = NPIX // 2
CH = C // 2             # 96 channels per core
HID = 384
TT = 512                # FFN token tile
T2 = 1024               # phase-1 token tile
SCALE = 1.0 / float(np.sqrt(C))
NCHUNK = 2              # AllGather chunks over channels
CCH = CH // NCHUNK

_NC_CACHE = {}


def build_nc(sim=False, dbg=False):
    nc = bacc.Bacc("TRN2", target_bir_lowering=False, debug=False,
                   num_devices=1 if sim else 8)

    x = nc.dram_tensor("x", [C + 2, NPIX], BF16, kind="ExternalInput")
    wq = nc.dram_tensor("wq", [97, 2 * CH], E4, kind="ExternalInput")
    wk = nc.dram_tensor("wk", [97, 2 * CH], E4, kind="ExternalInput")
    wv = nc.dram_tensor("wv", [97, 2 * CH], E4, kind="ExternalInput")
    wf1 = nc.dram_tensor("wf1", [C, HID], BF16, kind="ExternalInput")
    bf1c = nc.dram_tensor("bf1c", [HID, 1], F32, kind="ExternalInput")
    wf2 = nc.dram_tensor("wf2", [HID, C], BF16, kind="ExternalInput")
    bf2c = nc.dram_tensor("bf2c", [C, 1], F32, kind="ExternalInput")
    dyn = nc.dram_tensor("dyn", [1, 9], mybir.dt.uint32, kind="ExternalInput")
    out = nc.dram_tensor("out", [C, HNP], BF16, kind="ExternalOutput")

    # fp8 spills, natural local-channel rows.
    # qks layout per channel: (n2 128 block-pairs, j {q,k}, ip 512 tokens)
    qks = nc.dram_tensor("qks", [1, CH * 2 * NPIX], E4)
    vs = nc.dram_tensor("vs", [1, CH * NPIX], E3)  # plain (c,t)
    # x1s flat: block0 = peer-destined halves, block1 = own halves
    x1s = nc.dram_tensor("x1s", [1, 2 * CH * HNP], E3)
    x1gp = nc.dram_tensor("x1gp", [NCHUNK * 2 * CCH, HNP], E3)
    if dbg:
        qksb = nc.dram_tensor("qksb", [1, CH * 2 * NPIX], BF16,
                              kind="ExternalOutput")
        vsb = nc.dram_tensor("vsb", [1, CH * NPIX], BF16,
                             kind="ExternalOutput")
        x1sb = nc.dram_tensor("x1sb", [1, 2 * CH * HNP], BF16,
                              kind="ExternalOutput")
        x1gpb = nc.dram_tensor("x1gpb", [NCHUNK * 2 * CCH, HNP], BF16,
                               kind="ExternalOutput")
        xfb = nc.dram_tensor("xfb", [2 * C, HNP], BF16,
                             kind="ExternalOutput")

    xa_v = x.ap()
    xa_p = x.ap().rearrange("(r k) t -> r k t", k=2)   # 97 pair-rows
    # layout (n2, j, c, ip): per-write footprints are compact and
    # disjoint across token groups, so the dep tracker creates no false
    # WAW chain between spill DMAs (that chain paced all of phase 1).
    qks_wj = [qks.ap().rearrange("o (n2 j c ip) -> (o j) c n2 ip",
                                 n2=128, j=2, ip=512)[j] for j in range(2)]
    # read view: [cgroup, n2(part), j, (c ip)]
    qks_r = qks.ap().rearrange("o (n2 j cg cip) -> cg (o n2) j cip",
                               n2=128, j=2, cip=4 * 512)
    # vs layout (in1, c, p), same reasoning
    vs_w = vs.ap().rearrange("o (q c p) -> (o c) q p", c=CH, p=256)
    # read view: [cgroup, i, n1(part), (c p)]
    vs_r = vs.ap().rearrange("o (i n1 cg cip) -> cg i (o n1) cip",
                             i=2, n1=128, cip=4 * 256)
    x1s_r = x1s.ap().rearrange("o (r t) -> (o r) t", t=HNP)
    # per-channel-row view with the output-block partition dim leading:
    # row = block*CH + channel; [n 128, row, p 256]
    x1s_n = x1s.ap().rearrange("o (r n p) -> n (o r) p", n=128, p=256)
    x1gp_g = x1gp.ap().rearrange("(g rc) t -> g rc t", g=NCHUNK)
    x1gp_f = x1gp.ap()
    out_v = out.ap().rearrange("(g c) (w t) -> c g w t", g=2, t=TT)

    def with_track(a, off):
        return bass.AP(tensor=a.tensor, offset=a.offset, ap=a.ap,
                       const_val=a.const_val,
                       runtime_checks=a.runtime_checks,
                       dep_tracking_offset=off)

    with tile.TileContext(nc) as tc:
        # ---- persistent weights + own-half x cache ----
        with tc.tile_pool(name="wpool", bufs=1) as wp:
            ones_t = wp.tile([128, 1], E3, name="ones", tag="ones")
            nc.gpsimd.memset(ones_t[:], 1.0)
            w_t = {}
            for nm, wt in (("q", wq), ("k", wk), ("v", wv)):
                w_t[nm] = wp.tile([97, 2 * CH], E4, name=f"wt{nm}",
                                  tag=f"wt{nm}")
                nc.sync.dma_start(w_t[nm][:], wt.ap()[:, :])
            wf1_a = wp.tile([CH, HID], BF16, name="wf1a", tag="wf1a")
            wf1_b = wp.tile([CH, HID], BF16, name="wf1b", tag="wf1b")
            wf2_h = [wp.tile([128, C], BF16, name=f"wf2{hc}", tag=f"wf2{hc}")
                     for hc in range(3)]
            bf1_t = [wp.tile([128, 1], F32, name=f"bf1{hc}", tag=f"bf1{hc}")
                     for hc in range(3)]
            bf2_t = [wp.tile([CH, 1], F32, name=f"bf2{cc}", tag=f"bf2{cc}")
                     for cc in range(2)]

            def load_ffn_weights():
                nc.sync.dma_start(wf1_a[:], wf1.ap()[0:CH, :])
                nc.sync.dma_start(wf1_b[:], wf1.ap()[CH:C, :])
                for hc in range(3):
                    nc.sync.dma_start(wf2_h[hc][:],
                                      wf2.ap()[hc * 128:(hc + 1) * 128, :])
                    nc.sync.dma_start(bf1_t[hc][:],
                                      bf1c.ap()[hc * 128:(hc + 1) * 128, :])
                for cc in range(2):
                    nc.sync.dma_start(bf2_t[cc][:],
                                      bf2c.ap()[cc * CH:(cc + 1) * CH, :])
            dyn_sb = wp.tile([1, 9], mybir.dt.uint32, name="dyn", tag="dyn")
            nc.sync.dma_start(dyn_sb[:], dyn.ap()[:, :])
            # o_m[m]: x1s row base (block*CH) for image-half m, in row units
            o_m = [nc.values_load(dyn_sb[0:1, i:i + 1], min_val=0,
                                  max_val=CH,
                                  skip_runtime_bounds_check=True)
                   for i in range(2)]
            pb0 = nc.values_load(dyn_sb[0:1, 2:3], min_val=0, max_val=CCH,
                                 skip_runtime_bounds_check=True)
            tv = nc.values_load(dyn_sb[0:1, 3:4], min_val=0, max_val=HNP,
                                skip_runtime_bounds_check=True)
            tv2 = nc.values_load(dyn_sb[0:1, 4:5], min_val=0, max_val=HNP,
                                 skip_runtime_bounds_check=True)
            # same offsets in block-pair (512-token) units for the qk spill
            tvn = nc.values_load(dyn_sb[0:1, 5:6], min_val=0, max_val=64,
                                 skip_runtime_bounds_check=True)
            tv2n = nc.values_load(dyn_sb[0:1, 6:7], min_val=0, max_val=64,
                                  skip_runtime_bounds_check=True)
            # offsets in 256-token units for the vs spill
            tvp = nc.values_load(dyn_sb[0:1, 7:8], min_val=0, max_val=128,
                                 skip_runtime_bounds_check=True)
            tv2p = nc.values_load(dyn_sb[0:1, 8:9], min_val=0, max_val=128,
                                  skip_runtime_bounds_check=True)


            # ---- phase 1: QKV projections ----
            # token order: own half (t2 0..31) then peer half; 2-t2 batches
            with tc.tile_pool(name="px", bufs=8) as px, \
                 tc.tile_pool(name="pev", bufs=8) as pev, \
                 tc.tile_pool(name="psP", bufs=4, space="PSUM") as psP:
                for g2 in range(32):
                    own = g2 < 16
                    base = tv if own else tv2
                    basen = tvn if own else tv2n
                    basep = tvp if own else tv2p
                    goff = (g2 % 16) * (2 * T2)
                    o = base + goff
                    on = basen + (g2 % 16) * 4
                    op = basep + (g2 % 16) * 8
                    xe = px.tile([97, 2 * 2 * T2], E4, name="xe", tag="xe")
                    xev = xe[:].rearrange("r (k t) -> r k t", k=2)
                    srce = with_track(xa_p[:, :, bass.ds(o, 2 * T2)], goff)
                    nc.gpsimd.dma_start(xev, srce)
                    cqk = pev.tile([CH, 2 * 2 * T2], E4, name="cqk",
                                   tag="cqk")
                    cqkv = cqk[:].rearrange("c (j n2 ip) -> c j n2 ip",
                                            j=2, ip=512)
                    cv = pev.tile([CH, 2 * T2], E3, name="cv", tag="cv")
                    for hh in range(2):
                        hhs = slice(hh * T2, (hh + 1) * T2)
                        for j, nm in enumerate(("q", "k", "v")):
                            ps = psP.tile([CH, T2], F32, name="pp", tag="pp")
                            for q in range(2):
                                hs = slice(hh * T2 + q * TT,
                                           hh * T2 + (q + 1) * TT)
                                nc.tensor.matmul(
                                    ps[:, q * TT:(q + 1) * TT],
                                    w_t[nm][:].rearrange(
                                        "r (k m) -> r k m", k=2),
                                    xev[:, :, hs], start=True, stop=True,
                                    perf_mode=DR)
                            if j == 2:
                                dst_ = cv[:, hhs]
                            elif j == 1:
                                dst_ = cqk[:, 2048 + hh * T2:
                                           2048 + (hh + 1) * T2]
                            else:
                                dst_ = cqk[:, hh * T2:(hh + 1) * T2]
                            if (3 * hh + j) % 2 == 0:
                                nc.vector.tensor_copy(dst_, ps[:])
                            else:
                                nc.scalar.copy(dst_, ps[:])
                    # spills on HWDGE queues (sync/scalar): keeping them off
                    # the Pool queue stops their embedded evac-waits from
                    # head-of-line blocking the next groups' cast-loads.
                    for j in range(2):
                        dstq = with_track(
                            qks_wj[j][:, bass.ds(on, 4), :],
                            g2 * 4 * 2 * CH * 512)
                        nc.sync.dma_start(dstq, cqkv[:, j, :, :])
                    dstv = with_track(
                        vs_w[:, bass.ds(op, 8), :], g2 * 8 * CH * 256)
                    nc.scalar.dma_start(
                        dstv, cv[:].rearrange("c (q p) -> c q p", p=256))

            if dbg:
                nc.gpsimd.dma_start(qksb.ap()[0], qks.ap()[0])
                nc.gpsimd.dma_start(vsb.ap()[0], vs.ap()[0])


            # the spill writes use dynamic offsets the dependency tracker
            # cannot pair with phase 2's static reads -- hard barrier.
            tc.strict_bb_all_engine_barrier()

            # ---- phase 2: per-channel attention (+ overlapped AllGather) ---
            load_ffn_weights()
            with tc.tile_pool(name="aq", bufs=6) as aq, \
                 tc.tile_pool(name="ao", bufs=6) as ao, \
                 tc.tile_pool(name="ar", bufs=8) as ar, \
                 tc.tile_pool(name="psT", bufs=4, space="PSUM") as psT, \
                 tc.tile_pool(name="psU", bufs=4, space="PSUM") as psU:
                for cp2 in range(CH // 4):
                    # 4 consecutive channels per load
                    csl = slice(4 * cp2, 4 * cp2 + 4)
                    qkt = aq.tile([128, 4096], E4, name="qkt", tag="qkt")
                    qktv = qkt[:].rearrange("n (j ch w p) -> n j ch w p",
                                            ch=4, j=2, p=256)
                    nc.sync.dma_start(
                        qkt[:].rearrange("n (j cip) -> n j cip", j=2),
                        qks_r[cp2])
                    # ungapped V tile (i, ch, p): loads keep the source's
                    # 1KB contiguous runs (1x DMA rate); the softmax
                    # denominator comes from two N=1 matmuls vs a ones
                    # column instead of a gap-embedded ones column.
                    vv = aq.tile([128, 2048], E3, name="vv", tag="vv")
                    vvv = vv[:].rearrange("n (i ch p) -> n i ch p",
                                          ch=4, p=256)
                    for i in range(2):
                        nc.sync.dma_start(
                            vv[:, i * 1024:(i + 1) * 1024], vs_r[cp2, i])
                    for pr in range(2):
                        c0 = 4 * cp2 + 2 * pr  # first of 2 adjacent channels
                        ob4 = ao.tile([128, 1024], E3, name="ob4",
                                      tag="ob4")
                        ob4v = ob4[:].rearrange("n (u m p) -> n u m p",
                                                u=2, p=256)
                        for uu in range(2):
                            ch = 2 * pr + uu
                            tps = psT.tile([128, 512], F32, name="t",
                                           tag="t")
                            for j in range(2):
                                nc.tensor.matmul(
                                    tps[:, j * 256:(j + 1) * 256],
                                    qktv[:, 1, ch, :,
                                         j * 128:(j + 1) * 128],
                                    qktv[:, 0, ch, :, :],
                                    start=True, stop=True, perf_mode=DR)
                            te = ar.tile([128, 512], BF16, name="te",
                                         tag="te")
                            nc.scalar.activation(te[:], tps[:], AF.Exp,
                                                 scale=SCALE)
                            for m in range(2):
                                ups = psU.tile([128, 272], F32, name="u",
                                               tag="u")
                                for j in range(2):
                                    nc.tensor.matmul(
                                        ups[:, 0:256],
                                        te[:, j * 256 + m * 128:
                                           j * 256 + (m + 1) * 128],
                                        vvv[:, j, ch, :],
                                        start=(j == 0), stop=False,
                                        skip_group_check=True)
                                for j in range(2):
                                    nc.tensor.matmul(
                                        ups[:, 256:257],
                                        te[:, j * 256 + m * 128:
                                           j * 256 + (m + 1) * 128],
                                        ones_t[:],
                                        start=False, stop=(j == 1),
                                        skip_group_check=True)
                                rc = ar.tile([128, 1], F32, name="rc",
                                             tag="rc")
                                nc.vector.reciprocal(rc[:], ups[:, 256:257])
                                nc.vector.tensor_scalar_mul(
                                    ob4v[:, uu, m, :],
                                    ups[:, 0:256], rc[:])
                        # one DMA per image-half: 2 channels x [128, 256]
                        for m in range(2):
                            dst = x1s_n[:, bass.ds(o_m[m] + c0, 2), :]
                            dst = with_track(dst, c0 * HNP)
                            weng = nc.sync if m == 0 else nc.gpsimd
                            weng.dma_start(dst, ob4v[:, :, m, :])
                    # chunked exchange as soon as a channel group is done
                    cend = 4 * cp2 + 4
                    if cend % CCH == 0:
                        g = cend // CCH - 1
                        gsl = slice(g * CCH, (g + 1) * CCH)
                        src = x1s_r[gsl, :]
                        dst = x1gp_f[g * 2 * CCH:(g + 1) * 2 * CCH, :]
                        if sim:
                            dv = dst.rearrange("(r c) t -> r c t", r=2)
                            nc.sync.dma_start(dv[0], src)
                            nc.sync.dma_start(dv[1], src)
                        else:
                            nc.gpsimd.collective_compute(
                                "AllGather", mybir.AluOpType.bypass,
                                replica_groups=[[0, 1], [2, 3], [4, 5],
                                                [6, 7]],
                                ins=[src], outs=[dst],
                            )

            if dbg:
                nc.gpsimd.dma_start(x1sb.ap()[0], x1s.ap()[0])
                nc.gpsimd.dma_start(x1gpb.ap()[:, :], x1gp.ap()[:, :])

            # x1s writes use dynamic offsets; barrier before the FFN reads
            tc.strict_bb_all_engine_barrier()

            # ---- phase 3: FFN on my token half (2-TT groups) ----
            # software-pipelined: loads+adds 2 groups ahead, h-matmuls 1
            # group ahead, so no engine stalls on the cross-engine chain
            # adds(DVE/Pool) -> h(PE) -> gelu(ACT) -> y(PE) -> stt(DVE).
            NTG = NPIX // 2 // T2  # 32 groups of 1024 tokens
            with tc.tile_pool(name="fx", bufs=4) as fx, \
                 tc.tile_pool(name="fh", bufs=6) as fh, \
                 tc.tile_pool(name="fo", bufs=4) as fo, \
                 tc.tile_pool(name="psH", bufs=6, space="PSUM") as psH, \
                 tc.tile_pool(name="psY", bufs=2, space="PSUM") as psY:

                def p3_loads(tg):
                    toff = tg * T2
                    to0 = fx.tile([CH, T2], E3, name="to0", tag="to0")
                    src0 = with_track(
                        x1s_r[CH:2 * CH, toff:toff + T2], toff)
                    nc.sync.dma_start(to0[:], src0)
                    to1 = fx.tile([CH, T2], E3, name="to1", tag="to1")
                    for g in range(NCHUNK):
                        eng = nc.sync if g % 2 == 0 else nc.gpsimd
                        eng.dma_start(
                            to1[g * CCH:(g + 1) * CCH, :],
                            x1gp_g[g, bass.ds(pb0, CCH), toff:toff + T2])
                    xr0 = fx.tile([CH, T2], BF16, name="xr0", tag="xr0")
                    nc.sync.dma_start(
                        xr0[:], xa_v[0:CH, bass.ds(tv + toff, T2)])
                    xr1 = fx.tile([CH, T2], BF16, name="xr1", tag="xr1")
                    nc.scalar.dma_start(
                        xr1[:], xa_v[CH:C, bass.ds(tv + toff, T2)])
                    xf0 = fx.tile([CH, T2], BF16, name="xf0", tag="xf0")
                    nc.vector.tensor_add(xf0[:], to0[:], xr0[:])
                    xf1 = fx.tile([CH, T2], BF16, name="xf1", tag="xf1")
                    nc.gpsimd.tensor_add(xf1[:], to1[:], xr1[:])
                    if dbg:
                        for ti, t_ in ((0, to0), (1, to1), (2, xf0), (3, xf1)):
                            nc.gpsimd.dma_start(
                                xfb.ap()[ti * CH:(ti + 1) * CH,
                                         toff:toff + T2], t_[:])
                    return xf0, xf1

                def p3_h(tg, xfs):
                    xf0, xf1 = xfs
                    hsb = []
                    for s in range(2):
                        ssl = slice(s * TT, (s + 1) * TT)
                        for hc in range(3):
                            hcs = slice(hc * 128, (hc + 1) * 128)
                            hps = psH.tile([128, TT], F32, name="h", tag="h")
                            nc.tensor.matmul(hps[:], wf1_a[:, hcs],
                                             xf0[:, ssl],
                                             start=True, stop=False)
                            nc.tensor.matmul(hps[:], wf1_b[:, hcs],
                                             xf1[:, ssl],
                                             start=False, stop=True)
                            th = fh.tile([128, TT], BF16, name=f"h{hc}",
                                         tag=f"h{hc}")
                            nc.scalar.activation(th[:], hps[:], AF.Gelu,
                                                 bias=bf1_t[hc][:])
                            hsb.append(th)
                    return hsb

                def p3_tail(tg, xfs, hsb):
                    xf0, xf1 = xfs
                    oo = fo.tile([CH, 2 * 2 * TT], BF16, name="oo", tag="oo")
                    oov = oo[:].rearrange("c (g s t) -> c g s t", g=2, s=2)
                    for s in range(2):
                        ssl = slice(s * TT, (s + 1) * TT)
                        for cc in range(2):
                            ccs = slice(cc * CH, (cc + 1) * CH)
                            yps = psY.tile([CH, TT], F32, name="y", tag="y")
                            for hc in range(3):
                                nc.tensor.matmul(yps[:],
                                                 wf2_h[hc][:, ccs],
                                                 hsb[3 * s + hc][:],
                                                 start=(hc == 0),
                                                 stop=(hc == 2))
                            xf = xf0 if cc == 0 else xf1
                            nc.vector.scalar_tensor_tensor(
                                oov[:, cc, s, :], yps[:], bf2_t[cc][:],
                                xf[:, ssl], mybir.AluOpType.add,
                                mybir.AluOpType.add)
                    dsto = with_track(
                        out_v[:, :, 2 * tg:2 * tg + 2, :], tg * T2)
                    nc.sync.dma_start(dsto, oov)

                xq = {0: p3_loads(0), 1: p3_loads(1)}
                hq = {0: p3_h(0, xq[0])}
                for tg in range(NTG):
                    if tg + 2 < NTG:
                        xq[tg + 2] = p3_loads(tg + 2)
                    if tg + 1 < NTG:
                        hq[tg + 1] = p3_h(tg + 1, xq[tg + 1])
                    p3_tail(tg, xq.pop(tg), hq.pop(tg))
    nc.compile()
    return nc


def _get_nc():
    if "nc" not in _NC_CACHE:
        _NC_CACHE["nc"] = build_nc()
    return _NC_CACHE["nc"]


def _block(x):
    """(B,C,256,256) -> (B,C,65536) blocked token order."""
    Bn, Cn = x.shape[0], x.shape[1]
    return (x.reshape(Bn, Cn, 16, 16, 16, 16)
            .transpose(0, 1, 2, 4, 3, 5)
            .reshape(Bn, Cn, NPIX))


def _unblock(y):
    """(B,C,65536) blocked -> (B,C,256,256)."""
    Bn, Cn = y.shape[0], y.shape[1]
    return (y.reshape(Bn, Cn, 16, 16, 16, 16)
            .transpose(0, 1, 2, 4, 3, 5)
            .reshape(Bn, Cn, H, W))


def prepare_in_maps(x, Wq, bq, Wk, bk, Wv, bv, Wf1, bf1, Wf2, bf2):
    xb = _block(np.asarray(x, np.float32))
    xb_bf = xb.astype(ml_dtypes.bfloat16)
    ones = np.ones((1, NPIX), ml_dtypes.bfloat16)
    wf1_f = np.asarray(Wf1, np.float32)
    wf2_f = np.asarray(Wf2, np.float32)
    bf1_in = np.asarray(bf1, np.float32).reshape(HID, 1)
    bf2_f = np.asarray(bf2, np.float32)
    in_maps = []
    for k in range(8):
        b, h = k // 2, k % 2
        own = slice(h * CH, (h + 1) * CH)
        perm = np.r_[np.arange(h * CH, (h + 1) * CH),
                     np.arange((1 - h) * CH, (2 - h) * CH)]
        zrow = np.zeros((1, NPIX), ml_dtypes.bfloat16)
        x_in = np.concatenate([xb_bf[b][perm], ones, zrow], axis=0)
        wf1_in = np.ascontiguousarray(wf1_f[:, perm].T
                                      ).astype(ml_dtypes.bfloat16)
        wf2_in = np.ascontiguousarray(wf2_f[perm].T
                                      ).astype(ml_dtypes.bfloat16)
        bf2_in = bf2_f[perm].reshape(C, 1)
        # o_m in x1s row units: image-half m lands in block o (0=peer-
        # destined, 1=own); row base = o*CH
        tvv, tvv2 = h * HNP, (1 - h) * HNP
        dyn = np.array([[CH if h == 0 else 0, CH if h == 1 else 0,
                         (1 - h) * CCH, tvv, tvv2,
                         tvv // 512, tvv2 // 512,
                         tvv // 256, tvv2 // 256]], np.uint32)
        m = {"x": np.ascontiguousarray(x_in), "dyn": dyn,
             "wf1": wf1_in, "wf2": wf2_in, "bf1c": bf1_in, "bf2c": bf2_in}
        for nm, Wm, bm in (("wq", Wq, bq), ("wk", Wk, bk), ("wv", Wv, bv)):
            Wm = np.asarray(Wm, np.float32)
            wown = Wm[own]
            bown = np.asarray(bm, np.float32)[own]
            wext = np.concatenate([wown[:, perm].T, bown[None, :],
                                   np.zeros((1, CH), np.float32)], axis=0)
            m[nm] = np.ascontiguousarray(wext.reshape(97, 2 * CH)).astype(
                ml_dtypes.float8_e4m3)
        in_maps.append(m)
    return in_maps


def run(in_maps, trace=False, **kw):
    nc = _get_nc()
    return run_bass_kernel_spmd(nc, in_maps, core_ids=list(range(8)),
                                trace=trace, **kw)


def assemble(results):
    yb = np.empty((B, C, NPIX), np.float32)
    for k in range(8):
        b, h = k // 2, k % 2
        perm = np.r_[np.arange(h * CH, (h + 1) * CH),
                     np.arange((1 - h) * CH, (2 - h) * CH)]
        o = results[k]["out"]
        yb[b, perm, h * HNP:(h + 1) * HNP] = o.astype(np.float32)
    return _unblock(yb)


def kernel(**inputs):
    in_maps = prepare_in_maps(**inputs)
    res = run(in_maps)
    return assemble(res.results)
